# revision 4
# baseline (speedup 1.0000x reference)
"""Trainium2 Bass kernel for nn_HSR_2_25116968747549 (gnn_message_passing).

The reference's edge construction (`tile(B,1).reshape(2,-1)`, the preserved
index-mixing bug) makes `edge_src == edge_dst` for every edge: all edges are
self-edges, so each GATv2 layer collapses to the dense affine map
x -> (x @ Wl + bl + cb) @ linw and the whole network is

    t   = leaky_relu(x @ M1 + v1, 0.01)          M1 = Wl1@linw1@w1  (64x64)
    a   = rsqrt(mean(t^2) - mean(t)^2 + eps)     (per row; layernorm rstd)
    out = leaky_relu((a * t) @ M2c + v2, 0.01)   M2c = (I - J/64) diag(gamma) M2

Device layout (per core, 1024 rows): everything runs TRANSPOSED — features on
partitions, rows on the free dim.  The 1024 rows are stacked as two 64-
partition chunks (rows 0:512 -> partitions 0:64, rows 512:1024 -> 64:128) of
one [128, 512] tile, so each elementwise op covers the whole core's data in a
single full-width instruction and the two 64x64 matmuls per phase run
CONCURRENTLY in disjoint PE-array quadrants (tile_position packing).  Row
stats (the only partition-dim reduction) are computed with a [128,2] selector
matmul; the per-row scale is broadcast back across partitions with a [2,128]
selector matmul.  All matmul operands are bf16 (1 cycle/row vs 4 for fp32);
tolerance is 2e-2 so bf16 rounding (~0.3% norm-rel) is comfortably safe.
"""

import os

import numpy as np

B, W, D, H = 256, 32, 64, 4
N = B * W
NCORES = 8
RPC = N // NCORES          # rows per core = 1024
HALF = RPC // 2            # free dim = 512
EPS = 1e-5
SLOPE = 0.01

# Implementation variants (defaults = shipping config; env vars for tuning).
USE_PRELU = os.environ.get("K_PRELU", "1") == "1"       # ACT parametric_relu
USE_ABS_RSQRT = os.environ.get("K_ABSRSQRT", "1") == "1"  # ACT 1/sqrt(|x|)
OUT_BF16 = os.environ.get("K_OUTBF16", "1") == "1"


def _fold_weights(inp):
    f = lambda k: np.asarray(inp[k], np.float64)
    M1 = f("Wl1") @ f("linw1") @ f("w1")
    v1 = (f("bl1") + f("cb1")) @ f("linw1") @ f("w1") + f("b1")
    A2w = f("Wl2") @ f("linw2") @ f("w2")
    M2 = f("gamma")[:, None] * A2w
    v2 = f("beta") @ A2w + (f("bl2") + f("cb2")) @ f("linw2") @ f("w2") + f("b2")
    Cm = np.eye(D) - 1.0 / D
    M2c = Cm @ M2
    return M1, v1, M2c, v2


def _edges_degenerate(src, dst):
    src = np.asarray(src)
    dst = np.asarray(dst)
    return src.shape == dst.shape and np.array_equal(src, dst) and np.all(
        np.bincount(dst.astype(np.int64), minlength=N)[:N] > 0
    )


def _numpy_fallback(inp):
    # Generic (slow) host implementation, only used if the edge arrays ever
    # stop being fully degenerate.
    x = np.asarray(inp["x"], np.float32).reshape(N, D)
    src = np.asarray(inp["edge_src"]).astype(np.int64)
    dst = np.asarray(inp["edge_dst"]).astype(np.int64)

    def gat(xf, Wl, bl, Wr, br, att, cb, linw):
        xl = (xf @ Wl + bl).reshape(N, H, D)
        xr = (xf @ Wr + br).reshape(N, H, D)
        e = xl[src] + xr[dst]
        e = np.where(e > 0, e, 0.2 * e)
        logits = np.einsum("ehd,hd->eh", e, att)
        m = np.full((N, H), -np.inf, np.float32)
        np.maximum.at(m, dst, logits)
        ex = np.exp(logits - m[dst])
        den = np.zeros((N, H), np.float32)
        np.add.at(den, dst, ex)
        alpha = ex / den[dst]
        out = np.zeros((N, H, D), np.float32)
        np.add.at(out, dst, xl[src] * alpha[:, :, None])
        return (out.reshape(N, H * D) + cb) @ linw

    g = lambda k: np.asarray(inp[k], np.float32)
    lr = lambda t, a: np.where(t > 0, t, a * t)
    out = gat(x, g("Wl1"), g("bl1"), g("Wr1"), g("br1"), g("att1"), g("cb1"), g("linw1"))
    out = lr(out @ g("w1") + g("b1"), 0.01)
    mu = out.mean(-1, keepdims=True)
    var = ((out - mu) ** 2).mean(-1, keepdims=True)
    out = (out - mu) / np.sqrt(var + EPS) * g("gamma") + g("beta")
    out = gat(out, g("Wl2"), g("bl2"), g("Wr2"), g("br2"), g("att2"), g("cb2"), g("linw2"))
    out = lr(out @ g("w2") + g("b2"), 0.01)
    return out.reshape(B, W, D).astype(np.float32)


def build_bass():
    from concourse import bacc, mybir
    import concourse.tile as tile

    fp32 = mybir.dt.float32
    bf16 = mybir.dt.bfloat16
    out_dt = bf16 if OUT_BF16 else fp32
    Act = mybir.ActivationFunctionType
    Alu = mybir.AluOpType

    nc = bacc.Bacc()
    xs_d = nc.declare_dram_parameter("xs", [128, HALF], bf16, isOutput=False)
    wp_d = nc.declare_dram_parameter("wp", [128, 258], bf16, isOutput=False)
    vp_d = nc.declare_dram_parameter("vp", [128, 4], fp32, isOutput=False)
    ys_d = nc.declare_dram_parameter("ys", [128, HALF], out_dt, isOutput=True)

    with tile.TileContext(nc) as tc:
        with (
            tc.tile_pool(name="const", bufs=1) as cpool,
            tc.tile_pool(name="psum", bufs=1, space="PSUM") as ppool,
        ):
            xsb = cpool.tile([128, HALF], bf16, tag="xsb")
            wsb = cpool.tile([128, 258], bf16, tag="wsb")
            vsb = cpool.tile([128, 4], fp32, tag="vsb")
            t_sb = cpool.tile([128, HALF], bf16, tag="t")
            sq_sb = cpool.tile([128, HALF], bf16, tag="sq")
            msq_sb = cpool.tile([2, HALF], fp32, tag="msq")
            var_sb = cpool.tile([2, HALF], fp32, tag="var")
            a_sb = cpool.tile([2, HALF], bf16, tag="a")
            u_sb = cpool.tile([128, HALF], bf16, tag="u")
            z_sb = cpool.tile([128, HALF], bf16, tag="z")
            o_sb = cpool.tile([128, HALF], out_dt, tag="o")
            warm = cpool.tile([1, 1], fp32, tag="warm")

            P1 = ppool.tile([128, HALF], fp32, tag="P1")
            Sm = ppool.tile([2, HALF], fp32, tag="Sm")
            Sq = ppool.tile([2, HALF], fp32, tag="Sq")
            U = ppool.tile([128, HALF], fp32, tag="U")
            Bb = ppool.tile([128, HALF], fp32, tag="Bb")

            # ---- input DMAs (SP engine), issued first so they overlap
            # the ACT table load below.
            nc.sync.dma_start(out=xsb[:], in_=xs_d[:])
            nc.sync.dma_start(out=wsb[:], in_=wp_d[:])
            nc.sync.dma_start(out=vsb[:], in_=vp_d[:])

            # ACT table warm-up: force the single table containing every
            # func we use (parametric_relu/square/abs_reciprocal_sqrt/sqrt)
            # to load while the input DMA runs.
            nc.vector.memset(warm[:], 1.0)
            nc.scalar.activation(
                out=warm[:], in_=warm[:],
                func=(Act.Abs_reciprocal_sqrt if USE_ABS_RSQRT else Act.Sqrt),
                bias=0.0,
            )

            # ---- phase 1: tT = x @ M1 (transposed), two concurrent
            # 64x64-quadrant matmuls.
            nc.tensor.matmul(out=P1[0:64, :], lhsT=wsb[0:64, 0:64],
                             rhs=xsb[0:64, :], start=True, stop=True)
            nc.tensor.matmul(out=P1[64:128, :], lhsT=wsb[64:128, 0:64],
                             rhs=xsb[64:128, :], start=True, stop=True)

            # t = leaky_relu(tT + v1) -> bf16
            if USE_PRELU:
                nc.scalar.activation(
                    out=t_sb[:], in_=P1[:], func=Act.Prelu,
                    bias=vsb[:, 0:1], scale=1.0, alpha=SLOPE,
                )
            else:
                tf = cpool.tile([128, HALF], fp32, tag="tf")
                lp = cpool.tile([128, HALF], bf16, tag="lp")
                nc.vector.tensor_scalar(
                    out=tf[:], in0=P1[:], scalar1=vsb[:, 0:1], scalar2=None,
                    op0=Alu.add,
                )
                nc.vector.tensor_scalar(
                    out=lp[:], in0=tf[:], scalar1=SLOPE, scalar2=None,
                    op0=Alu.mult,
                )
                nc.vector.tensor_tensor(
                    out=t_sb[:], in0=lp[:], in1=tf[:], op=Alu.max,
                )

            # sq = t*t (DVE bf16 fast mode)
            nc.vector.tensor_tensor(out=sq_sb[:], in0=t_sb[:], in1=t_sb[:],
                                    op=Alu.mult)

            # row means: selector matmul, sel2 columns carry 1/64 blocks so
            # psum receives means directly.  Sm[0,:]=chunk0, Sm[1,:]=chunk1.
            nc.tensor.matmul(out=Sm[0:2, :], lhsT=wsb[:, 128:130],
                             rhs=t_sb[:], start=True, stop=True)
            nc.tensor.matmul(out=Sq[0:2, :], lhsT=wsb[:, 128:130],
                             rhs=sq_sb[:], start=True, stop=True)

            # u = t @ M2c (unscaled; the per-row scale commutes past M2c and
            # is applied afterwards).  Two concurrent quadrant matmuls.
            nc.tensor.matmul(out=U[0:64, :], lhsT=wsb[0:64, 64:128],
                             rhs=t_sb[0:64, :], start=True, stop=True)
            nc.tensor.matmul(out=U[64:128, :], lhsT=wsb[64:128, 64:128],
                             rhs=t_sb[64:128, :], start=True, stop=True)

            # u -> sbuf bf16 (GPSIMD cannot read PSUM on trn2; DVE has an
            # idle window here during the rsqrt/broadcast stages).
            nc.vector.tensor_copy(out=u_sb[:], in_=U[0:128, :])

            # stats: var = E[t^2] - E[t]^2 ; a = rsqrt(var + eps)
            nc.scalar.activation(out=msq_sb[:], in_=Sm[0:2, :],
                                 func=Act.Square, bias=vsb[0:2, 3:4])
            nc.vector.scalar_tensor_tensor(
                out=var_sb[:], in0=msq_sb[:], scalar=-1.0, in1=Sq[0:2, :],
                op0=Alu.mult, op1=Alu.add,
            )
            if USE_ABS_RSQRT:
                nc.scalar.activation(
                    out=a_sb[:], in_=var_sb[:], func=Act.Abs_reciprocal_sqrt,
                    bias=vsb[0:2, 2:3],
                )
            else:
                sd_sb = cpool.tile([2, HALF], fp32, tag="sd")
                nc.scalar.activation(out=sd_sb[:], in_=var_sb[:],
                                     func=Act.Sqrt, bias=vsb[0:2, 2:3])
                nc.vector.reciprocal(out=a_sb[:], in_=sd_sb[:])

            # broadcast a across partitions: Bb[p,j] = a[chunk(p), j]
            nc.tensor.matmul(out=Bb[0:128, :], lhsT=wsb[0:2, 130:258],
                             rhs=a_sb[:], start=True, stop=True)

            # z = a * u
            nc.vector.scalar_tensor_tensor(
                out=z_sb[:], in0=Bb[0:128, :], scalar=1.0, in1=u_sb[:],
                op0=Alu.mult, op1=Alu.mult,
            )

            # out = leaky_relu(z + v2)
            if USE_PRELU:
                nc.scalar.activation(
                    out=o_sb[:], in_=z_sb[:], func=Act.Prelu,
                    bias=vsb[:, 1:2], scale=1.0, alpha=SLOPE,
                )
            else:
                zf = cpool.tile([128, HALF], fp32, tag="zf")
                lp2 = cpool.tile([128, HALF], bf16, tag="lp2")
                nc.vector.tensor_scalar(
                    out=zf[:], in0=z_sb[:], scalar1=vsb[:, 1:2], scalar2=None,
                    op0=Alu.add,
                )
                nc.vector.tensor_scalar(
                    out=lp2[:], in0=zf[:], scalar1=SLOPE, scalar2=None,
                    op0=Alu.mult,
                )
                nc.vector.tensor_tensor(
                    out=o_sb[:], in0=lp2[:], in1=zf[:], op=Alu.max,
                )

            # output DMA issued by the ACT engine itself (saves a hop).
            nc.scalar.dma_start(out=ys_d[:], in_=o_sb[:])

    return nc


MAX_SEM = os.environ.get("K_MAXSEM", "78")


def _patch_walrus_max_sems():
    """Cap walrus's semaphore allocation.  The NEFF epilogue zeroes every
    semaphore in the kernel range one instruction at a time (~250 instrs,
    ~6-7us of measured exec time); shrinking the pool shrinks that teardown.
    78 is the documented minimum (see env.get_walrus_max_sem_num)."""
    if not MAX_SEM:
        return
    from concourse import bass_utils

    if getattr(bass_utils, "_max_sem_patched", False):
        return
    orig = bass_utils.run_command

    def patched(argv, **kwargs):
        if argv and "walrus_driver" in str(argv[0]) and "codegen" in ",".join(
            str(a) for a in argv
        ):
            argv = list(argv) + [f"--max-sem-num={MAX_SEM}"]
        return orig(argv, **kwargs)

    bass_utils.run_command = patched
    bass_utils._max_sem_patched = True


def kernel(**inputs):
    if not _edges_degenerate(inputs["edge_src"], inputs["edge_dst"]):
        return _numpy_fallback(inputs)

    import ml_dtypes
    from concourse.bass_utils import run_bass_kernel_spmd

    _patch_walrus_max_sems()

    bf16 = ml_dtypes.bfloat16
    M1, v1, M2c, v2 = _fold_weights(inputs)

    wpack = np.zeros((128, 258), np.float32)
    wpack[0:64, 0:64] = M1
    wpack[64:128, 0:64] = M1
    wpack[0:64, 64:128] = M2c
    wpack[64:128, 64:128] = M2c
    wpack[0:64, 128] = 1.0 / D          # sel2 col0: chunk0 mean
    wpack[64:128, 129] = 1.0 / D        # sel2 col1: chunk1 mean
    wpack[0, 130:130 + 64] = 1.0        # selBT row0 -> partitions 0:64
    wpack[1, 130 + 64:258] = 1.0        # selBT row1 -> partitions 64:128
    wpack = wpack.astype(bf16)

    vpack = np.zeros((128, 4), np.float32)
    vpack[0:64, 0] = v1
    vpack[64:128, 0] = v1
    vpack[0:64, 1] = v2
    vpack[64:128, 1] = v2
    vpack[:, 2] = EPS
    # col 3 stays zero (Square bias)

    xf = np.asarray(inputs["x"], np.float32).reshape(N, D)
    in_maps = []
    for c in range(NCORES):
        xc = xf[c * RPC:(c + 1) * RPC]
        xst = np.concatenate([xc[0:HALF].T, xc[HALF:].T], 0)  # [128, 512]
        in_maps.append({
            "xs": np.ascontiguousarray(xst).astype(bf16),
            "wp": wpack,
            "vp": vpack,
        })

    nc = build_bass()
    if not nc.is_finalized():
        nc.finalize()
    res = run_bass_kernel_spmd(nc, in_maps, list(range(NCORES)))
    global LAST_RESULT
    LAST_RESULT = res
    outs = []
    for r in res.results:
        ys = np.asarray(r["ys"], np.float32)          # [128, 512]
        outs.append(ys[0:64].T)                        # rows 0:512
        outs.append(ys[64:128].T)                      # rows 512:1024
    return np.concatenate(outs, 0).reshape(B, W, D).astype(np.float32)


LAST_RESULT = None


if __name__ == "__main__":
    print("kernel module ok")


# revision 7
# speedup vs baseline: 1.0188x; 1.0188x over previous
"""Trainium2 Bass kernel for nn_HSR_2_25116968747549 (gnn_message_passing).

The reference's edge construction (`tile(B,1).reshape(2,-1)`, the preserved
index-mixing bug) makes `edge_src == edge_dst` for every edge: all edges are
self-edges, so each GATv2 layer collapses to the dense affine map
x -> (x @ Wl + bl + cb) @ linw and the whole network is

    t   = leaky_relu(x @ M1 + v1, 0.01)          M1 = Wl1@linw1@w1  (64x64)
    a   = rsqrt(mean(t^2) - mean(t)^2 + eps)     (per row; layernorm rstd)
    out = leaky_relu((a * t) @ M2c + v2, 0.01)   M2c = (I - J/64) diag(gamma) M2

Device layout (per core, 1024 rows): everything runs TRANSPOSED — features on
partitions, rows on the free dim.  The 1024 rows are stacked as two 64-
partition chunks (rows 0:512 -> partitions 0:64, rows 512:1024 -> 64:128) of
one [128, 512] tile, so each elementwise op covers the whole core's data in a
single full-width instruction and the two 64x64 matmuls per phase run
CONCURRENTLY in disjoint PE-array quadrants (tile_position packing).  Row
stats (the only partition-dim reduction) are computed with a [128,2] selector
matmul; the per-row scale is broadcast back across partitions with a [2,128]
selector matmul.  All matmul operands are bf16 (1 cycle/row vs 4 for fp32);
tolerance is 2e-2 so bf16 rounding (~0.3% norm-rel) is comfortably safe.
"""

import os

import numpy as np

B, W, D, H = 256, 32, 64, 4
N = B * W
NCORES = 8
RPC = N // NCORES          # rows per core = 1024
HALF = RPC // 2            # free dim = 512
EPS = 1e-5
SLOPE = 0.01

# Implementation variants (defaults = shipping config; env vars for tuning).
USE_PRELU = os.environ.get("K_PRELU", "1") == "1"       # ACT parametric_relu
USE_ABS_RSQRT = os.environ.get("K_ABSRSQRT", "1") == "1"  # ACT 1/sqrt(|x|)
OUT_BF16 = os.environ.get("K_OUTBF16", "1") == "1"


def _fold_weights(inp):
    f = lambda k: np.asarray(inp[k], np.float64)
    M1 = f("Wl1") @ f("linw1") @ f("w1")
    v1 = (f("bl1") + f("cb1")) @ f("linw1") @ f("w1") + f("b1")
    A2w = f("Wl2") @ f("linw2") @ f("w2")
    M2 = f("gamma")[:, None] * A2w
    v2 = f("beta") @ A2w + (f("bl2") + f("cb2")) @ f("linw2") @ f("w2") + f("b2")
    Cm = np.eye(D) - 1.0 / D
    M2c = Cm @ M2
    return M1, v1, M2c, v2


def _edges_degenerate(src, dst):
    src = np.asarray(src)
    dst = np.asarray(dst)
    return src.shape == dst.shape and np.array_equal(src, dst) and np.all(
        np.bincount(dst.astype(np.int64), minlength=N)[:N] > 0
    )


def _numpy_fallback(inp):
    # Generic (slow) host implementation, only used if the edge arrays ever
    # stop being fully degenerate.
    x = np.asarray(inp["x"], np.float32).reshape(N, D)
    src = np.asarray(inp["edge_src"]).astype(np.int64)
    dst = np.asarray(inp["edge_dst"]).astype(np.int64)

    def gat(xf, Wl, bl, Wr, br, att, cb, linw):
        xl = (xf @ Wl + bl).reshape(N, H, D)
        xr = (xf @ Wr + br).reshape(N, H, D)
        e = xl[src] + xr[dst]
        e = np.where(e > 0, e, 0.2 * e)
        logits = np.einsum("ehd,hd->eh", e, att)
        m = np.full((N, H), -np.inf, np.float32)
        np.maximum.at(m, dst, logits)
        ex = np.exp(logits - m[dst])
        den = np.zeros((N, H), np.float32)
        np.add.at(den, dst, ex)
        alpha = ex / den[dst]
        out = np.zeros((N, H, D), np.float32)
        np.add.at(out, dst, xl[src] * alpha[:, :, None])
        return (out.reshape(N, H * D) + cb) @ linw

    g = lambda k: np.asarray(inp[k], np.float32)
    lr = lambda t, a: np.where(t > 0, t, a * t)
    out = gat(x, g("Wl1"), g("bl1"), g("Wr1"), g("br1"), g("att1"), g("cb1"), g("linw1"))
    out = lr(out @ g("w1") + g("b1"), 0.01)
    mu = out.mean(-1, keepdims=True)
    var = ((out - mu) ** 2).mean(-1, keepdims=True)
    out = (out - mu) / np.sqrt(var + EPS) * g("gamma") + g("beta")
    out = gat(out, g("Wl2"), g("bl2"), g("Wr2"), g("br2"), g("att2"), g("cb2"), g("linw2"))
    out = lr(out @ g("w2") + g("b2"), 0.01)
    return out.reshape(B, W, D).astype(np.float32)


def build_bass():
    from concourse import bacc, mybir
    import concourse.tile as tile

    fp32 = mybir.dt.float32
    bf16 = mybir.dt.bfloat16
    out_dt = bf16 if OUT_BF16 else fp32
    Act = mybir.ActivationFunctionType
    Alu = mybir.AluOpType

    nc = bacc.Bacc()
    xs_d = nc.declare_dram_parameter("xs", [128, HALF], bf16, isOutput=False)
    wp_d = nc.declare_dram_parameter("wp", [128, 258], bf16, isOutput=False)
    vp_d = nc.declare_dram_parameter("vp", [128, 4], fp32, isOutput=False)
    ys_d = nc.declare_dram_parameter("ys", [128, HALF], out_dt, isOutput=True)

    with tile.TileContext(nc) as tc:
        with (
            tc.tile_pool(name="const", bufs=1) as cpool,
            tc.tile_pool(name="psum", bufs=1, space="PSUM") as ppool,
        ):
            xsb = cpool.tile([128, HALF], bf16, tag="xsb")
            wsb = cpool.tile([128, 258], bf16, tag="wsb")
            vsb = cpool.tile([128, 4], fp32, tag="vsb")
            t_sb = cpool.tile([128, HALF], bf16, tag="t")
            sq_sb = cpool.tile([128, HALF], bf16, tag="sq")
            msq_sb = cpool.tile([2, HALF], fp32, tag="msq")
            var_sb = cpool.tile([2, HALF], fp32, tag="var")
            a_sb = cpool.tile([2, HALF], bf16, tag="a")
            u_sb = cpool.tile([128, HALF], bf16, tag="u")
            z_sb = cpool.tile([128, HALF], bf16, tag="z")
            o_sb = cpool.tile([128, HALF], out_dt, tag="o")
            warm = cpool.tile([1, 1], fp32, tag="warm")

            P1 = ppool.tile([128, HALF], fp32, tag="P1")
            Sm = ppool.tile([2, HALF], fp32, tag="Sm")
            Sq = ppool.tile([34, HALF], fp32, tag="Sq")
            U = ppool.tile([128, HALF], fp32, tag="U")
            Bb = ppool.tile([128, HALF], fp32, tag="Bb")

            # ---- input DMAs (SP engine), issued first so they overlap
            # the ACT table load below.
            nc.sync.dma_start(out=xsb[:], in_=xs_d[:])
            nc.sync.dma_start(out=wsb[:], in_=wp_d[:])
            nc.sync.dma_start(out=vsb[:], in_=vp_d[:])

            # ACT table warm-up: force the single table containing every
            # func we use (parametric_relu/square/abs_reciprocal_sqrt/sqrt)
            # to load while the input DMA runs.
            nc.vector.memset(warm[:], 1.0)
            nc.scalar.activation(
                out=warm[:], in_=warm[:],
                func=(Act.Abs_reciprocal_sqrt if USE_ABS_RSQRT else Act.Sqrt),
                bias=0.0,
            )

            # ---- phase 1: tT = x @ M1 (transposed), two concurrent
            # 64x64-quadrant matmuls.
            nc.tensor.matmul(out=P1[0:64, :], lhsT=wsb[0:64, 0:64],
                             rhs=xsb[0:64, :], start=True, stop=True)
            nc.tensor.matmul(out=P1[64:128, :], lhsT=wsb[64:128, 0:64],
                             rhs=xsb[64:128, :], start=True, stop=True)

            # t = leaky_relu(tT + v1) -> bf16
            if USE_PRELU:
                nc.scalar.activation(
                    out=t_sb[:], in_=P1[:], func=Act.Prelu,
                    bias=vsb[:, 0:1], scale=1.0, alpha=SLOPE,
                )
            else:
                tf = cpool.tile([128, HALF], fp32, tag="tf")
                lp = cpool.tile([128, HALF], bf16, tag="lp")
                nc.vector.tensor_scalar(
                    out=tf[:], in0=P1[:], scalar1=vsb[:, 0:1], scalar2=None,
                    op0=Alu.add,
                )
                nc.vector.tensor_scalar(
                    out=lp[:], in0=tf[:], scalar1=SLOPE, scalar2=None,
                    op0=Alu.mult,
                )
                nc.vector.tensor_tensor(
                    out=t_sb[:], in0=lp[:], in1=tf[:], op=Alu.max,
                )

            # sq = t*t (DVE bf16 fast mode)
            nc.vector.tensor_tensor(out=sq_sb[:], in0=t_sb[:], in1=t_sb[:],
                                    op=Alu.mult)

            # row means: selector matmul, sel2 columns carry 1/64 blocks so
            # psum receives means directly.  Sm[0,:]=chunk0, Sm[1,:]=chunk1.
            # Sq goes to array column-strip 1 (out partitions 32:34) so the
            # two selector matmuls occupy disjoint PE sub-arrays and run
            # concurrently.
            nc.tensor.matmul(out=Sm[0:2, :], lhsT=wsb[:, 128:130],
                             rhs=t_sb[:], start=True, stop=True)
            nc.tensor.matmul(out=Sq[32:34, :], lhsT=wsb[:, 128:130],
                             rhs=sq_sb[:], start=True, stop=True,
                             tile_position=(0, 32))

            # u = t @ M2c (unscaled; the per-row scale commutes past M2c and
            # is applied afterwards).  Two concurrent quadrant matmuls.
            # Emitted after the stats matmuls: u is not needed until z.
            nc.tensor.matmul(out=U[0:64, :], lhsT=wsb[0:64, 64:128],
                             rhs=t_sb[0:64, :], start=True, stop=True)
            nc.tensor.matmul(out=U[64:128, :], lhsT=wsb[64:128, 64:128],
                             rhs=t_sb[64:128, :], start=True, stop=True)

            # stats: var = E[t^2] - E[t]^2 ; a = rsqrt(var + eps)
            nc.scalar.activation(out=msq_sb[:], in_=Sm[0:2, :],
                                 func=Act.Square, bias=vsb[0:2, 3:4])
            nc.vector.scalar_tensor_tensor(
                out=var_sb[:], in0=msq_sb[:], scalar=-1.0, in1=Sq[32:34, :],
                op0=Alu.mult, op1=Alu.add,
            )

            # u -> sbuf bf16 (GPSIMD cannot read PSUM on trn2).  Emitted
            # after var so the DVE FIFO runs sq -> var -> u_copy -> z; the
            # copy fills the DVE idle window during rsqrt + broadcast.
            nc.vector.tensor_copy(out=u_sb[:], in_=U[0:128, :])
            if USE_ABS_RSQRT:
                nc.scalar.activation(
                    out=a_sb[:], in_=var_sb[:], func=Act.Abs_reciprocal_sqrt,
                    bias=vsb[0:2, 2:3],
                )
            else:
                sd_sb = cpool.tile([2, HALF], fp32, tag="sd")
                nc.scalar.activation(out=sd_sb[:], in_=var_sb[:],
                                     func=Act.Sqrt, bias=vsb[0:2, 2:3])
                nc.vector.reciprocal(out=a_sb[:], in_=sd_sb[:])

            # broadcast a across partitions: Bb[p,j] = a[chunk(p), j]
            nc.tensor.matmul(out=Bb[0:128, :], lhsT=wsb[0:2, 130:258],
                             rhs=a_sb[:], start=True, stop=True)

            # z = a * u
            nc.vector.scalar_tensor_tensor(
                out=z_sb[:], in0=Bb[0:128, :], scalar=1.0, in1=u_sb[:],
                op0=Alu.mult, op1=Alu.mult,
            )

            # out = leaky_relu(z + v2)
            if USE_PRELU:
                nc.scalar.activation(
                    out=o_sb[:], in_=z_sb[:], func=Act.Prelu,
                    bias=vsb[:, 1:2], scale=1.0, alpha=SLOPE,
                )
            else:
                zf = cpool.tile([128, HALF], fp32, tag="zf")
                lp2 = cpool.tile([128, HALF], bf16, tag="lp2")
                nc.vector.tensor_scalar(
                    out=zf[:], in0=z_sb[:], scalar1=vsb[:, 1:2], scalar2=None,
                    op0=Alu.add,
                )
                nc.vector.tensor_scalar(
                    out=lp2[:], in0=zf[:], scalar1=SLOPE, scalar2=None,
                    op0=Alu.mult,
                )
                nc.vector.tensor_tensor(
                    out=o_sb[:], in0=lp2[:], in1=zf[:], op=Alu.max,
                )

            # output DMA issued by the ACT engine itself (saves a hop).
            nc.scalar.dma_start(out=ys_d[:], in_=o_sb[:])

    return nc


def kernel(**inputs):
    if not _edges_degenerate(inputs["edge_src"], inputs["edge_dst"]):
        return _numpy_fallback(inputs)

    import ml_dtypes
    from concourse.bass_utils import run_bass_kernel_spmd

    bf16 = ml_dtypes.bfloat16
    M1, v1, M2c, v2 = _fold_weights(inputs)

    wpack = np.zeros((128, 258), np.float32)
    wpack[0:64, 0:64] = M1
    wpack[64:128, 0:64] = M1
    wpack[0:64, 64:128] = M2c
    wpack[64:128, 64:128] = M2c
    wpack[0:64, 128] = 1.0 / D          # sel2 col0: chunk0 mean
    wpack[64:128, 129] = 1.0 / D        # sel2 col1: chunk1 mean
    wpack[0, 130:130 + 64] = 1.0        # selBT row0 -> partitions 0:64
    wpack[1, 130 + 64:258] = 1.0        # selBT row1 -> partitions 64:128
    wpack = wpack.astype(bf16)

    vpack = np.zeros((128, 4), np.float32)
    vpack[0:64, 0] = v1
    vpack[64:128, 0] = v1
    vpack[0:64, 1] = v2
    vpack[64:128, 1] = v2
    vpack[:, 2] = EPS
    # col 3 stays zero (Square bias)

    xf = np.asarray(inputs["x"], np.float32).reshape(N, D)
    in_maps = []
    for c in range(NCORES):
        xc = xf[c * RPC:(c + 1) * RPC]
        xst = np.concatenate([xc[0:HALF].T, xc[HALF:].T], 0)  # [128, 512]
        in_maps.append({
            "xs": np.ascontiguousarray(xst).astype(bf16),
            "wp": wpack,
            "vp": vpack,
        })

    nc = build_bass()
    if not nc.is_finalized():
        nc.finalize()
    res = run_bass_kernel_spmd(nc, in_maps, list(range(NCORES)))
    global LAST_RESULT
    LAST_RESULT = res
    outs = []
    for r in res.results:
        ys = np.asarray(r["ys"], np.float32)          # [128, 512]
        outs.append(ys[0:64].T)                        # rows 0:512
        outs.append(ys[64:128].T)                      # rows 512:1024
    return np.concatenate(outs, 0).reshape(B, W, D).astype(np.float32)


LAST_RESULT = None


if __name__ == "__main__":
    print("kernel module ok")


# revision 9
# speedup vs baseline: 1.0273x; 1.0083x over previous
"""Trainium2 Bass kernel for nn_HSR_2_25116968747549 (gnn_message_passing).

The reference's edge construction (`tile(B,1).reshape(2,-1)`, the preserved
index-mixing bug) makes `edge_src == edge_dst` for every edge: all edges are
self-edges, so each GATv2 layer collapses to the dense affine map
x -> (x @ Wl + bl + cb) @ linw and the whole network is

    t   = leaky_relu(x @ M1 + v1, 0.01)          M1 = Wl1@linw1@w1  (64x64)
    a   = rsqrt(mean(t^2) - mean(t)^2 + eps)     (per row; layernorm rstd)
    out = leaky_relu((a * t) @ M2c + v2, 0.01)   M2c = (I - J/64) diag(gamma) M2

Device layout (per core, 1024 rows): everything runs TRANSPOSED — features on
partitions, rows on the free dim.  The 1024 rows are stacked as two 64-
partition chunks (rows 0:512 -> partitions 0:64, rows 512:1024 -> 64:128) of
one [128, 512] tile, so each elementwise op covers the whole core's data in a
single full-width instruction and the two 64x64 matmuls per phase run
CONCURRENTLY in disjoint PE-array quadrants (tile_position packing).  Row
stats (the only partition-dim reduction) are computed with a [128,2] selector
matmul; the per-row scale is broadcast back across partitions with a [2,128]
selector matmul.  All matmul operands are bf16 (1 cycle/row vs 4 for fp32);
tolerance is 2e-2 so bf16 rounding (~0.3% norm-rel) is comfortably safe.
"""

import os

import numpy as np

B, W, D, H = 256, 32, 64, 4
N = B * W
NCORES = 8
RPC = N // NCORES          # rows per core = 1024
HALF = RPC // 2            # free dim = 512
EPS = 1e-5
SLOPE = 0.01

# Implementation variants (defaults = shipping config; env vars for tuning).
USE_PRELU = os.environ.get("K_PRELU", "1") == "1"       # ACT parametric_relu
USE_ABS_RSQRT = os.environ.get("K_ABSRSQRT", "1") == "1"  # ACT 1/sqrt(|x|)
OUT_BF16 = os.environ.get("K_OUTBF16", "1") == "1"


def _fold_weights(inp):
    f = lambda k: np.asarray(inp[k], np.float64)
    M1 = f("Wl1") @ f("linw1") @ f("w1")
    v1 = (f("bl1") + f("cb1")) @ f("linw1") @ f("w1") + f("b1")
    A2w = f("Wl2") @ f("linw2") @ f("w2")
    M2 = f("gamma")[:, None] * A2w
    v2 = f("beta") @ A2w + (f("bl2") + f("cb2")) @ f("linw2") @ f("w2") + f("b2")
    Cm = np.eye(D) - 1.0 / D
    M2c = Cm @ M2
    return M1, v1, M2c, v2


def _edges_degenerate(src, dst):
    src = np.asarray(src)
    dst = np.asarray(dst)
    return src.shape == dst.shape and np.array_equal(src, dst) and np.all(
        np.bincount(dst.astype(np.int64), minlength=N)[:N] > 0
    )


def _numpy_fallback(inp):
    # Generic (slow) host implementation, only used if the edge arrays ever
    # stop being fully degenerate.
    x = np.asarray(inp["x"], np.float32).reshape(N, D)
    src = np.asarray(inp["edge_src"]).astype(np.int64)
    dst = np.asarray(inp["edge_dst"]).astype(np.int64)

    def gat(xf, Wl, bl, Wr, br, att, cb, linw):
        xl = (xf @ Wl + bl).reshape(N, H, D)
        xr = (xf @ Wr + br).reshape(N, H, D)
        e = xl[src] + xr[dst]
        e = np.where(e > 0, e, 0.2 * e)
        logits = np.einsum("ehd,hd->eh", e, att)
        m = np.full((N, H), -np.inf, np.float32)
        np.maximum.at(m, dst, logits)
        ex = np.exp(logits - m[dst])
        den = np.zeros((N, H), np.float32)
        np.add.at(den, dst, ex)
        alpha = ex / den[dst]
        out = np.zeros((N, H, D), np.float32)
        np.add.at(out, dst, xl[src] * alpha[:, :, None])
        return (out.reshape(N, H * D) + cb) @ linw

    g = lambda k: np.asarray(inp[k], np.float32)
    lr = lambda t, a: np.where(t > 0, t, a * t)
    out = gat(x, g("Wl1"), g("bl1"), g("Wr1"), g("br1"), g("att1"), g("cb1"), g("linw1"))
    out = lr(out @ g("w1") + g("b1"), 0.01)
    mu = out.mean(-1, keepdims=True)
    var = ((out - mu) ** 2).mean(-1, keepdims=True)
    out = (out - mu) / np.sqrt(var + EPS) * g("gamma") + g("beta")
    out = gat(out, g("Wl2"), g("bl2"), g("Wr2"), g("br2"), g("att2"), g("cb2"), g("linw2"))
    out = lr(out @ g("w2") + g("b2"), 0.01)
    return out.reshape(B, W, D).astype(np.float32)


def build_bass():
    from concourse import bacc, mybir
    import concourse.tile as tile

    fp32 = mybir.dt.float32
    bf16 = mybir.dt.bfloat16
    out_dt = bf16 if OUT_BF16 else fp32
    Act = mybir.ActivationFunctionType
    Alu = mybir.AluOpType

    nc = bacc.Bacc()
    xs_d = nc.declare_dram_parameter("xs", [128, HALF], bf16, isOutput=False)
    wp_d = nc.declare_dram_parameter("wp", [128, 258], bf16, isOutput=False)
    vp_d = nc.declare_dram_parameter("vp", [128, 4], fp32, isOutput=False)
    ys_d = nc.declare_dram_parameter("ys", [128, HALF], out_dt, isOutput=True)

    with tile.TileContext(nc) as tc:
        with (
            tc.tile_pool(name="const", bufs=1) as cpool,
            tc.tile_pool(name="psum", bufs=1, space="PSUM") as ppool,
        ):
            xsb = cpool.tile([128, HALF], bf16, tag="xsb")
            wsb = cpool.tile([128, 258], bf16, tag="wsb")
            vsb = cpool.tile([128, 4], fp32, tag="vsb")
            t_sb = cpool.tile([128, HALF], bf16, tag="t")
            sq_sb = cpool.tile([128, HALF], bf16, tag="sq")
            msq_sb = cpool.tile([2, HALF], fp32, tag="msq")
            var_sb = cpool.tile([2, HALF], fp32, tag="var")
            a_sb = cpool.tile([2, HALF], bf16, tag="a")
            u_sb = cpool.tile([128, HALF], bf16, tag="u")
            z_sb = cpool.tile([128, HALF], bf16, tag="z")
            o_sb = cpool.tile([128, HALF], out_dt, tag="o")
            warm = cpool.tile([1, 1], fp32, tag="warm")

            P1 = ppool.tile([128, HALF], fp32, tag="P1")
            Sm = ppool.tile([2, HALF], fp32, tag="Sm")
            Sq = ppool.tile([34, HALF], fp32, tag="Sq")
            U = ppool.tile([128, HALF], fp32, tag="U")
            Bb = ppool.tile([128, HALF], fp32, tag="Bb")

            # ---- input DMAs (SP engine), issued first so they overlap
            # the ACT table load below.
            nc.sync.dma_start(out=xsb[:], in_=xs_d[:])
            nc.sync.dma_start(out=wsb[:], in_=wp_d[:])
            nc.sync.dma_start(out=vsb[:], in_=vp_d[:])

            # ACT table warm-up: force the single table containing every
            # func we use (parametric_relu/square/abs_reciprocal_sqrt/sqrt)
            # to load while the input DMA runs.
            nc.vector.memset(warm[:], 1.0)
            nc.scalar.activation(
                out=warm[:], in_=warm[:],
                func=(Act.Abs_reciprocal_sqrt if USE_ABS_RSQRT else Act.Sqrt),
                bias=0.0,
            )

            # ---- phase 1: tT = x @ M1 (transposed), two concurrent
            # 64x64-quadrant matmuls.
            nc.tensor.matmul(out=P1[0:64, :], lhsT=wsb[0:64, 0:64],
                             rhs=xsb[0:64, :], start=True, stop=True)
            nc.tensor.matmul(out=P1[64:128, :], lhsT=wsb[64:128, 0:64],
                             rhs=xsb[64:128, :], start=True, stop=True)

            # t = leaky_relu(tT + v1) -> bf16
            if USE_PRELU:
                nc.scalar.activation(
                    out=t_sb[:], in_=P1[:], func=Act.Prelu,
                    bias=vsb[:, 0:1], scale=1.0, alpha=SLOPE,
                )
            else:
                tf = cpool.tile([128, HALF], fp32, tag="tf")
                lp = cpool.tile([128, HALF], bf16, tag="lp")
                nc.vector.tensor_scalar(
                    out=tf[:], in0=P1[:], scalar1=vsb[:, 0:1], scalar2=None,
                    op0=Alu.add,
                )
                nc.vector.tensor_scalar(
                    out=lp[:], in0=tf[:], scalar1=SLOPE, scalar2=None,
                    op0=Alu.mult,
                )
                nc.vector.tensor_tensor(
                    out=t_sb[:], in0=lp[:], in1=tf[:], op=Alu.max,
                )

            # sq = t*t (DVE bf16 fast mode)
            nc.vector.tensor_tensor(out=sq_sb[:], in0=t_sb[:], in1=t_sb[:],
                                    op=Alu.mult)

            # row means: selector matmul, sel2 columns carry 1/64 blocks so
            # psum receives means directly.  Sm[0,:]=chunk0, Sm[1,:]=chunk1.
            # Sq goes to array column-strip 1 (out partitions 32:34) so the
            # two selector matmuls occupy disjoint PE sub-arrays and run
            # concurrently.
            nc.tensor.matmul(out=Sm[0:2, :], lhsT=wsb[:, 128:130],
                             rhs=t_sb[:], start=True, stop=True)
            nc.tensor.matmul(out=Sq[32:34, :], lhsT=wsb[:, 128:130],
                             rhs=sq_sb[:], start=True, stop=True,
                             tile_position=(0, 32))

            # u = t @ M2c (unscaled; the per-row scale commutes past M2c and
            # is applied afterwards).  Two concurrent quadrant matmuls.
            # tile_wait_until keeps the scheduler from hoisting these ahead
            # of the latency-critical sums_sq matmul in the PE queue (u is
            # not needed until z, ~3us later).
            with tc.tile_wait_until(0.0055):
                nc.tensor.matmul(out=U[0:64, :], lhsT=wsb[0:64, 64:128],
                                 rhs=t_sb[0:64, :], start=True, stop=True)
                nc.tensor.matmul(out=U[64:128, :], lhsT=wsb[64:128, 64:128],
                                 rhs=t_sb[64:128, :], start=True, stop=True)

            # stats: var = E[t^2] - E[t]^2 ; a = rsqrt(var + eps)
            nc.scalar.activation(out=msq_sb[:], in_=Sm[0:2, :],
                                 func=Act.Square, bias=vsb[0:2, 3:4])
            nc.vector.scalar_tensor_tensor(
                out=var_sb[:], in0=msq_sb[:], scalar=-1.0, in1=Sq[32:34, :],
                op0=Alu.mult, op1=Alu.add,
            )

            # u -> sbuf bf16 (GPSIMD cannot read PSUM on trn2).  The wait
            # keeps it behind var in the DVE FIFO; it then fills the DVE
            # idle window during rsqrt + broadcast, well before z needs it.
            with tc.tile_wait_until(0.0063):
                nc.vector.tensor_copy(out=u_sb[:], in_=U[0:128, :])
            if USE_ABS_RSQRT:
                nc.scalar.activation(
                    out=a_sb[:], in_=var_sb[:], func=Act.Abs_reciprocal_sqrt,
                    bias=vsb[0:2, 2:3],
                )
            else:
                sd_sb = cpool.tile([2, HALF], fp32, tag="sd")
                nc.scalar.activation(out=sd_sb[:], in_=var_sb[:],
                                     func=Act.Sqrt, bias=vsb[0:2, 2:3])
                nc.vector.reciprocal(out=a_sb[:], in_=sd_sb[:])

            # broadcast a across partitions: Bb[p,j] = a[chunk(p), j]
            nc.tensor.matmul(out=Bb[0:128, :], lhsT=wsb[0:2, 130:258],
                             rhs=a_sb[:], start=True, stop=True)

            # z = a * u
            nc.vector.scalar_tensor_tensor(
                out=z_sb[:], in0=Bb[0:128, :], scalar=1.0, in1=u_sb[:],
                op0=Alu.mult, op1=Alu.mult,
            )

            # out = leaky_relu(z + v2)
            if USE_PRELU:
                nc.scalar.activation(
                    out=o_sb[:], in_=z_sb[:], func=Act.Prelu,
                    bias=vsb[:, 1:2], scale=1.0, alpha=SLOPE,
                )
            else:
                zf = cpool.tile([128, HALF], fp32, tag="zf")
                lp2 = cpool.tile([128, HALF], bf16, tag="lp2")
                nc.vector.tensor_scalar(
                    out=zf[:], in0=z_sb[:], scalar1=vsb[:, 1:2], scalar2=None,
                    op0=Alu.add,
                )
                nc.vector.tensor_scalar(
                    out=lp2[:], in0=zf[:], scalar1=SLOPE, scalar2=None,
                    op0=Alu.mult,
                )
                nc.vector.tensor_tensor(
                    out=o_sb[:], in0=lp2[:], in1=zf[:], op=Alu.max,
                )

            # output DMA issued by the ACT engine itself (saves a hop).
            nc.scalar.dma_start(out=ys_d[:], in_=o_sb[:])

    return nc


def kernel(**inputs):
    if not _edges_degenerate(inputs["edge_src"], inputs["edge_dst"]):
        return _numpy_fallback(inputs)

    import ml_dtypes
    from concourse.bass_utils import run_bass_kernel_spmd

    bf16 = ml_dtypes.bfloat16
    M1, v1, M2c, v2 = _fold_weights(inputs)

    wpack = np.zeros((128, 258), np.float32)
    wpack[0:64, 0:64] = M1
    wpack[64:128, 0:64] = M1
    wpack[0:64, 64:128] = M2c
    wpack[64:128, 64:128] = M2c
    wpack[0:64, 128] = 1.0 / D          # sel2 col0: chunk0 mean
    wpack[64:128, 129] = 1.0 / D        # sel2 col1: chunk1 mean
    wpack[0, 130:130 + 64] = 1.0        # selBT row0 -> partitions 0:64
    wpack[1, 130 + 64:258] = 1.0        # selBT row1 -> partitions 64:128
    wpack = wpack.astype(bf16)

    vpack = np.zeros((128, 4), np.float32)
    vpack[0:64, 0] = v1
    vpack[64:128, 0] = v1
    vpack[0:64, 1] = v2
    vpack[64:128, 1] = v2
    vpack[:, 2] = EPS
    # col 3 stays zero (Square bias)

    xf = np.asarray(inputs["x"], np.float32).reshape(N, D)
    in_maps = []
    for c in range(NCORES):
        xc = xf[c * RPC:(c + 1) * RPC]
        xst = np.concatenate([xc[0:HALF].T, xc[HALF:].T], 0)  # [128, 512]
        in_maps.append({
            "xs": np.ascontiguousarray(xst).astype(bf16),
            "wp": wpack,
            "vp": vpack,
        })

    nc = build_bass()
    if not nc.is_finalized():
        nc.finalize()
    res = run_bass_kernel_spmd(nc, in_maps, list(range(NCORES)))
    global LAST_RESULT
    LAST_RESULT = res
    outs = []
    for r in res.results:
        ys = np.asarray(r["ys"], np.float32)          # [128, 512]
        outs.append(ys[0:64].T)                        # rows 0:512
        outs.append(ys[64:128].T)                      # rows 512:1024
    return np.concatenate(outs, 0).reshape(B, W, D).astype(np.float32)


LAST_RESULT = None


if __name__ == "__main__":
    print("kernel module ok")
